# revision 7
# baseline (speedup 1.0000x reference)
"""Trainium2 Bass kernel for nn_Attention_v3 (sparse_attention).

Computes, per (b, n) group of 256 tokens:
    xn  = LayerNorm(x) * g_norm
    q   = xn @ W_q ; k, v = split(xn @ W_kv)
    sim = (q k^T) * scale ; attn = softmax(sim * 128)
    out = LayerNorm((attn @ v) @ W_out) * g_out

Device strategy (8 NeuronCores, data-parallel over the 256 (b, n) groups,
32 groups per core, processed in pairs):
  - softmax's row-max subtraction is a mathematical no-op here
    (softmax((s - m) * a) == softmax(s * a)) and the final LayerNorm is
    invariant to per-row positive scaling, so the softmax denominator
    cancels; softmax reduces to a bare elementwise exp on the ACT engine
    with the 1/sqrt(d) fold into its free scale slot.
  - attention is computed fully transposed (sim^T = k q^T per head) so no
    transposes of the attention matrix are ever needed.
  - matmuls run in float32r (full PE rate, ~1.6e-4 rel err) except the
    attn @ v stage, which uses bf16 inputs so the two heads of an output
    partition chunk can be packed with col-tiling (fp32 matmuls cannot
    write PSUM at partition base 64 on this toolchain).
"""

import numpy as np

B, N, R, DIM = 4, 64, 256, 512
HEADS, DH = 8, 64
NCORES = 8
GROUPS = B * N                 # 256
GPC = GROUPS // NCORES         # 32 groups per core
PAIRS = GPC // 2               # 16 pair iterations per core
EPS = 1e-5
EXP_SCALE = float(DH ** -0.5)  # SCALE * PB_ALPHA = 0.125

_BUILD_CACHE = {}


def _build_nc(apply_gout: bool, debug: bool = False):
    import concourse.bacc as bacc
    import concourse.mybir as mybir
    import concourse.tile as tile
    import concourse.bass as bass
    from concourse.masks import make_identity

    F32 = mybir.dt.float32
    F32R = mybir.dt.float32r
    BF16 = mybir.dt.bfloat16
    AF = mybir.ActivationFunctionType
    OP = mybir.AluOpType

    nc = bacc.Bacc("TRN2", target_bir_lowering=False, debug=False)

    x_d = nc.dram_tensor("x", [GPC * R, DIM], F32, kind="ExternalInput")
    gn_d = nc.dram_tensor("g_norm", [DIM], F32, kind="ExternalInput")
    wq_d = nc.dram_tensor("W_q", [DIM, DIM], F32R, kind="ExternalInput")
    wkv_d = nc.dram_tensor("W_kv", [DIM, 2 * DH], F32R, kind="ExternalInput")
    wo_d = nc.dram_tensor("W_out", [DIM, DIM], F32R, kind="ExternalInput")
    go_d = nc.dram_tensor("g_out", [DIM], F32, kind="ExternalInput")
    y_d = nc.dram_tensor("y", [GPC * R, DIM], F32, kind="ExternalOutput")
    dbg = {}
    if debug:
        F32R_ = mybir.dt.float32r
        BF16_ = mybir.dt.bfloat16
        dbg["xn"] = nc.dram_tensor("dbg_xn", [128, 4, DIM], F32R_, kind="ExternalOutput")
        dbg["xnT"] = nc.dram_tensor("dbg_xnT", [128, 4, DIM], F32R_, kind="ExternalOutput")
        dbg["qT"] = nc.dram_tensor("dbg_qT", [128, 4, DIM], F32R_, kind="ExternalOutput")
        dbg["kk"] = nc.dram_tensor("dbg_kk", [128, DIM], F32R_, kind="ExternalOutput")
        dbg["vT"] = nc.dram_tensor("dbg_vT", [128, DIM], F32R_, kind="ExternalOutput")
        dbg["v"] = nc.dram_tensor("dbg_v", [128, 4, 64], BF16_, kind="ExternalOutput")
        dbg["at"] = nc.dram_tensor("dbg_at", [4, 128, 8, 256], BF16_, kind="ExternalOutput")
        dbg["outT"] = nc.dram_tensor("dbg_outT", [128, 4, DIM], F32R_, kind="ExternalOutput")

    with tile.TileContext(nc) as tc:
        with (
            tc.tile_pool(name="consts", bufs=1) as consts,
            tc.tile_pool(name="work", bufs=2) as work,
            tc.tile_pool(name="attn", bufs=8) as attnp,
            tc.tile_pool(name="stats", bufs=8) as statsp,
            tc.tile_pool(name="ps", bufs=2, space="PSUM") as ps,
        ):
            # ---- constants / weights (once) ----
            ident_f = consts.tile([128, 128], F32)
            make_identity(nc, ident_f)
            ident = consts.tile([128, 128], F32R)
            nc.vector.tensor_copy(ident[:], ident_f[:])
            eps_t = consts.tile([128, 1], F32)
            nc.vector.memset(eps_t[:], EPS)

            gn_sb = consts.tile([128, 4], F32)
            nc.sync.dma_start(gn_sb[:], gn_d.ap().rearrange("(dc p) -> p dc", p=128))

            wq_sb = consts.tile([128, 4, DIM], F32R)
            nc.sync.dma_start(
                wq_sb[:], wq_d.ap().rearrange("(dc p) i -> p dc i", p=128)
            )
            wkv_sb = consts.tile([128, 4, 2 * DH], F32R)
            nc.sync.dma_start(
                wkv_sb[:], wkv_d.ap().rearrange("(dc p) k -> p dc k", p=128)
            )
            wo_sb = consts.tile([128, 4, DIM], F32R)
            nc.sync.dma_start(
                wo_sb[:], wo_d.ap().rearrange("(ic p) o -> p ic o", p=128)
            )
            # fold g_norm into the projection weights (rows of W scale by g)
            for dc in range(4):
                nc.vector.tensor_scalar_mul(
                    wq_sb[:, dc, :], wq_sb[:, dc, :], gn_sb[:, dc : dc + 1]
                )
                nc.vector.tensor_scalar_mul(
                    wkv_sb[:, dc, :], wkv_sb[:, dc, :], gn_sb[:, dc : dc + 1]
                )
            # W_k duplicated on both column halves (k^T lands on both
            # partition halves for row-tiled sim matmuls); W_v is a view.
            wkd_sb = consts.tile([128, 4, 128], F32R)
            nc.vector.tensor_copy(wkd_sb[:, :, 0:64], wkv_sb[:, :, 0:64])
            nc.vector.tensor_copy(wkd_sb[:, :, 64:128], wkv_sb[:, :, 0:64])

            ones_f = consts.tile([128, 64], F32)
            nc.vector.memset(ones_f[:], 1.0)
            ones_b = consts.tile([128, 64], BF16)
            nc.vector.tensor_copy(ones_b[:], ones_f[:])

            if apply_gout:
                go_sb = consts.tile([128, DIM], F32)
                go_ap = go_d.ap()
                nc.sync.dma_start(
                    go_sb[:],
                    bass.AP(tensor=go_ap.tensor, offset=go_ap.offset,
                            ap=[[0, 128]] + list(go_ap.ap)),
                )

            x_view = x_d.ap().rearrange("(p2 t p) d -> p2 p t d", p=128, t=4)
            y_view = y_d.ap().rearrange("(p2 t p) d -> p2 p t d", p=128, t=4)

            for pr in range(PAIRS):
                # ---- load + LayerNorm 1 ----
                xin = work.tile([128, 4, DIM], F32)
                nc.sync.dma_start(xin[:], x_view[pr])
                xn = work.tile([128, 4, DIM], F32R)
                for t in range(4):
                    st = statsp.tile([128, 6], F32)
                    nc.vector.bn_stats(st[:], xin[:, t, :])
                    mv = statsp.tile([128, 2], F32)
                    nc.vector.bn_aggr(mv[:], st[:])
                    rstd = statsp.tile([128, 1], F32)
                    nc.scalar.activation(
                        out=rstd[:], in_=mv[:, 1:2], func=AF.Sqrt,
                        bias=eps_t[:], scale=1.0,
                    )
                    nc.vector.reciprocal(out=rstd[:], in_=rstd[:])
                    nc.vector.tensor_scalar(
                        out=xn[:, t, :], in0=xin[:, t, :],
                        scalar1=mv[:, 0:1], scalar2=rstd[:],
                        op0=OP.subtract, op1=OP.mult,
                    )

                # ---- transpose xn -> xnT [dim, tok] ----
                stage = ps.tile([128, 4, 4, 128], F32R, tag="ps")
                for dc in range(4):
                    for t in range(4):
                        nc.tensor.transpose(
                            stage[:, dc, t, :],
                            xn[:, t, dc * 128 : (dc + 1) * 128],
                            ident[:],
                        )
                xnT = work.tile([128, 4, DIM], F32R)
                nc.vector.tensor_copy(xnT[:], stage[:])

                # ---- q^T ----
                qTp = ps.tile([128, 4, DIM], F32, tag="ps")
                for c in range(4):
                    for dc in range(4):
                        nc.tensor.matmul(
                            qTp[:, c, :],
                            wq_sb[:, dc, c * 128 : (c + 1) * 128],
                            xnT[:, dc, :],
                            start=(dc == 0), stop=(dc == 3),
                        )
                qT = work.tile([128, 4, DIM], F32R)
                nc.vector.tensor_copy(qT[:], qTp[:])

                # ---- k^T (duplicated) and v^T, then v ----
                kvp = ps.tile([128, 4, DIM], F32, tag="ps")
                for dc in range(4):
                    nc.tensor.matmul(
                        kvp[:, 0, :], wkd_sb[:, dc, :], xnT[:, dc, :],
                        start=(dc == 0), stop=(dc == 3),
                    )
                for dc in range(4):
                    nc.tensor.matmul(
                        kvp[0:64, 1, :],
                        wkv_sb[:, dc, 64:128],
                        xnT[:, dc, :],
                        start=(dc == 0), stop=(dc == 3),
                    )
                kk = work.tile([128, DIM], F32R)
                nc.vector.tensor_copy(kk[:], kvp[:, 0, :])
                vT = work.tile([128, DIM], F32R)
                nc.vector.tensor_copy(vT[0:64, :], kvp[0:64, 1, :])

                vtp = ps.tile([128, 4, 64], F32R, tag="ps")
                for s in range(4):  # s = g*2 + jc
                    nc.tensor.transpose(
                        vtp[:, s, :],
                        vT[0:64, s * 128 : (s + 1) * 128],
                        ident[0:64, 0:64],
                    )
                v_sb = work.tile([128, 4, 64], BF16)
                nc.vector.tensor_copy(v_sb[:], vtp[:])

                # ---- sim^T + exp per (group, j-chunk) ----
                attnT = [[None, None], [None, None]]
                for g in range(2):
                    for jc in range(2):
                        simp = ps.tile([128, 8, 256], F32, tag="ps")
                        for c in range(4):
                            for hp in range(2):
                                sl = hp * 4 + c
                                nc.tensor.matmul(
                                    simp[:, sl, :],
                                    kk[hp * 64 : hp * 64 + 64,
                                       g * 256 + jc * 128 : g * 256 + jc * 128 + 128],
                                    qT[hp * 64 : hp * 64 + 64, c,
                                       g * 256 : (g + 1) * 256],
                                    start=True, stop=True,
                                    tile_position=(hp * 64, 0),
                                )
                        at = attnp.tile([128, 8, 256], BF16, tag="attnT")
                        nc.scalar.activation(
                            out=at[:], in_=simp[:], func=AF.Exp, scale=EXP_SCALE,
                        )
                        attnT[g][jc] = at

                # ---- attn @ v (transposed out), bf16 col-tiled ----
                avp = ps.tile([128, 2, 4, 256], F32, tag="ps")
                for g in range(2):
                    for ic in range(4):
                        for jc in range(2):
                            for hp in range(2):
                                sl = hp * 4 + ic
                                nc.tensor.matmul(
                                    avp[hp * 64 : hp * 64 + 64, g, ic, :],
                                    v_sb[:, g * 2 + jc, :],
                                    attnT[g][jc][:, sl, :],
                                    start=(jc == 0), stop=(jc == 1),
                                    tile_position=(0, hp * 64),
                                )
                dnp = ps.tile([128, 2, 4, 256], F32, tag="ps")
                for g in range(2):
                    for ic in range(4):
                        for jc in range(2):
                            for hp in range(2):
                                sl = hp * 4 + ic
                                nc.tensor.matmul(
                                    dnp[hp * 64 : hp * 64 + 64, g, ic, :],
                                    ones_b[:, :],
                                    attnT[g][jc][:, sl, :],
                                    start=(jc == 0), stop=(jc == 1),
                                    tile_position=(0, hp * 64),
                                )
                dn_sb = work.tile([128, 2, 4, 256], F32)
                nc.vector.reciprocal(out=dn_sb[:], in_=dnp[:])
                outT = work.tile([128, 4, DIM], F32R)
                for g in range(2):
                    nc.vector.tensor_tensor(
                        outT[:, :, g * 256 : (g + 1) * 256],
                        avp[:, g, :, :], dn_sb[:, g, :, :], OP.mult,
                    )

                # ---- out projection + LayerNorm 2 ----
                finp = ps.tile([128, 4, DIM], F32, tag="ps")
                for t in range(4):
                    for ic in range(4):
                        nc.tensor.matmul(
                            finp[:, t, :],
                            outT[:, ic, t * 128 : (t + 1) * 128],
                            wo_sb[:, ic, :],
                            start=(ic == 0), stop=(ic == 3),
                        )
                y_sb = work.tile([128, 4, DIM], F32)
                for t in range(4):
                    st2 = statsp.tile([128, 6], F32)
                    nc.vector.bn_stats(st2[:], finp[:, t, :])
                    mv2 = statsp.tile([128, 2], F32)
                    nc.vector.bn_aggr(mv2[:], st2[:])
                    rstd2 = statsp.tile([128, 1], F32)
                    nc.scalar.activation(
                        out=rstd2[:], in_=mv2[:, 1:2], func=AF.Sqrt,
                        bias=eps_t[:], scale=1.0,
                    )
                    nc.vector.reciprocal(out=rstd2[:], in_=rstd2[:])
                    nc.vector.tensor_scalar(
                        out=y_sb[:, t, :], in0=finp[:, t, :],
                        scalar1=mv2[:, 0:1], scalar2=rstd2[:],
                        op0=OP.subtract, op1=OP.mult,
                    )
                    if apply_gout:
                        nc.vector.tensor_tensor(
                            y_sb[:, t, :], y_sb[:, t, :], go_sb[:], OP.mult
                        )
                nc.sync.dma_start(y_view[pr], y_sb[:])
                if debug and pr == 0:
                    nc.sync.dma_start(dbg["xn"].ap(), xn[:])
                    nc.sync.dma_start(dbg["xnT"].ap(), xnT[:])
                    nc.sync.dma_start(dbg["qT"].ap(), qT[:])
                    nc.sync.dma_start(dbg["kk"].ap(), kk[:])
                    nc.sync.dma_start(dbg["vT"].ap(), vT[:])
                    nc.sync.dma_start(dbg["v"].ap(), v_sb[:])
                    for g in range(2):
                        for jc in range(2):
                            nc.sync.dma_start(dbg["at"].ap()[g * 2 + jc], attnT[g][jc][:])
                    nc.sync.dma_start(dbg["outT"].ap(), outT[:])

    nc.finalize()
    return nc


def kernel(x, g_norm, W_q, W_kv, W_out, g_out):
    from concourse.bass_utils import run_bass_kernel_spmd

    x = np.ascontiguousarray(np.asarray(x, dtype=np.float32))
    g_norm = np.asarray(g_norm, dtype=np.float32)
    W_q = np.ascontiguousarray(np.asarray(W_q, dtype=np.float32))
    W_kv = np.ascontiguousarray(np.asarray(W_kv, dtype=np.float32))
    W_out = np.ascontiguousarray(np.asarray(W_out, dtype=np.float32))
    g_out = np.asarray(g_out, dtype=np.float32)

    apply_gout = not np.allclose(g_out, 1.0)
    key = apply_gout
    if key not in _BUILD_CACHE:
        _BUILD_CACHE[key] = _build_nc(apply_gout)
    nc = _BUILD_CACHE[key]

    xs = x.reshape(GROUPS, R, DIM)
    in_maps = []
    for c in range(NCORES):
        in_maps.append({
            "x": np.ascontiguousarray(
                xs[c * GPC : (c + 1) * GPC].reshape(GPC * R, DIM)
            ),
            "g_norm": g_norm,
            "W_q": W_q,
            "W_kv": W_kv,
            "W_out": W_out,
            "g_out": g_out,
        })

    res = run_bass_kernel_spmd(nc, in_maps, core_ids=list(range(NCORES)))
    y = np.empty((GROUPS, R, DIM), dtype=np.float32)
    for c in range(NCORES):
        y[c * GPC : (c + 1) * GPC] = res.results[c]["y"].reshape(GPC, R, DIM)
    return y.reshape(B, N, R, DIM)


# revision 24
# speedup vs baseline: 45.2443x; 45.2443x over previous
"""Trainium2 Bass kernel for nn_Attention_v3 (sparse_attention).

Per (b, n) group of 256 tokens:
    xn  = LayerNorm(x) * g_norm
    q   = xn @ W_q ; k, v = split(xn @ W_kv)
    attn = softmax(((q k^T) * scale - rowmax) * 128)
    out = LayerNorm((attn @ v) @ W_out) * g_out

Strategy (8 NeuronCores, data-parallel over the 256 (b, n) groups, 32 per
core, processed in pairs of groups so matmul free dims reach 512):
  - softmax's row-max subtraction is a mathematical no-op here
    (softmax((s - m) * a) == softmax(s * a)) and the scores are small, so
    the numerator is a bare exp on the ACT engine with the 1/sqrt(d) scale
    folded into the activation's free scale slot.
  - attention runs fully transposed (sim^T = k q^T per head, row-tiled
    over head parity pairs), so the attention matrix never needs a
    transpose; per-(head, token) softmax denominators come from a
    col-tiled ones-matmul that mirrors attn @ v and land in PSUM aligned
    with it, then reciprocal + multiply normalize the heads.
  - matmuls run in float32r (full PE rate at out free-dim >= 256,
    ~2e-4 rel err vs fp32's 4x slowdown); attn @ v and the ones-matmul
    use bf16 inputs because fp32-family matmuls cannot write PSUM at
    partition base 64, which the col-tiled head packing needs.
"""

import numpy as np

B, N, R, DIM = 4, 64, 256, 512
HEADS, DH = 8, 64
NCORES = 8
GROUPS = B * N                 # 256
GPC = GROUPS // NCORES         # 32 groups per core
PAIRS = GPC // 2               # 16 pair iterations per core
EPS = 1e-5
EXP_SCALE = float(DH ** -0.5)  # SCALE * PB_ALPHA = 0.125

_BUILD_CACHE = {}


def _build_nc(apply_gout: bool, debug: bool = False):
    import concourse.bacc as bacc
    import concourse.mybir as mybir
    import concourse.tile as tile
    import concourse.bass as bass
    from concourse.masks import make_identity

    F32 = mybir.dt.float32
    F32R = mybir.dt.float32r
    BF16 = mybir.dt.bfloat16
    AF = mybir.ActivationFunctionType
    OP = mybir.AluOpType

    nc = bacc.Bacc("TRN2", target_bir_lowering=False, debug=False)

    x_d = nc.dram_tensor("x", [GPC * R, DIM], F32, kind="ExternalInput")
    gn_d = nc.dram_tensor("g_norm", [DIM], F32, kind="ExternalInput")
    wq_d = nc.dram_tensor("W_q", [DIM, DIM], F32R, kind="ExternalInput")
    wkv_d = nc.dram_tensor("W_kv", [DIM, 2 * DH], F32R, kind="ExternalInput")
    wo_d = nc.dram_tensor("W_out", [DIM, DIM], F32R, kind="ExternalInput")
    go_d = nc.dram_tensor("g_out", [DIM], F32, kind="ExternalInput")
    y_d = nc.dram_tensor("y", [GPC * R, DIM], F32, kind="ExternalOutput")
    dbg = {}
    if debug:
        F32R_ = mybir.dt.float32r
        BF16_ = mybir.dt.bfloat16
        dbg["xn"] = nc.dram_tensor("dbg_xn", [128, 4, DIM], F32R_, kind="ExternalOutput")
        dbg["xnT"] = nc.dram_tensor("dbg_xnT", [128, 4, DIM], F32R_, kind="ExternalOutput")
        dbg["qT"] = nc.dram_tensor("dbg_qT", [128, 4, DIM], F32R_, kind="ExternalOutput")
        dbg["kk"] = nc.dram_tensor("dbg_kk", [128, DIM], F32R_, kind="ExternalOutput")
        dbg["vT"] = nc.dram_tensor("dbg_vT", [128, DIM], F32R_, kind="ExternalOutput")
        dbg["v"] = nc.dram_tensor("dbg_v", [128, 4, 64], BF16_, kind="ExternalOutput")
        dbg["at"] = nc.dram_tensor("dbg_at", [4, 128, 8, 256], BF16_, kind="ExternalOutput")
        dbg["outT"] = nc.dram_tensor("dbg_outT", [128, 4, DIM], F32R_, kind="ExternalOutput")

    with tile.TileContext(nc) as tc:
        with (
            tc.tile_pool(name="consts", bufs=1) as consts,
            tc.tile_pool(name="work", bufs=2) as work,
            tc.tile_pool(name="attn", bufs=8) as attnp,
            tc.tile_pool(name="stats", bufs=8) as statsp,
            tc.tile_pool(name="ps", bufs=2, space="PSUM") as ps,
        ):
            # ---- constants / weights (once) ----
            ident_f = consts.tile([128, 128], F32)
            make_identity(nc, ident_f)
            ident = consts.tile([128, 128], F32R)
            nc.vector.tensor_copy(ident[:], ident_f[:])
            eps_t = consts.tile([128, 1], F32)
            nc.vector.memset(eps_t[:], EPS)

            gn_sb = consts.tile([128, 4], F32)
            nc.sync.dma_start(gn_sb[:], gn_d.ap().rearrange("(dc p) -> p dc", p=128))

            wq_sb = consts.tile([128, 4, DIM], F32R)
            nc.sync.dma_start(
                wq_sb[:], wq_d.ap().rearrange("(dc p) i -> p dc i", p=128)
            )
            wkv_sb = consts.tile([128, 4, 2 * DH], F32R)
            nc.sync.dma_start(
                wkv_sb[:], wkv_d.ap().rearrange("(dc p) k -> p dc k", p=128)
            )
            wo_sb = consts.tile([128, 4, DIM], F32R)
            nc.sync.dma_start(
                wo_sb[:], wo_d.ap().rearrange("(ic p) o -> p ic o", p=128)
            )
            # fold g_norm into the projection weights (rows of W scale by g)
            for dc in range(4):
                nc.vector.tensor_scalar_mul(
                    wq_sb[:, dc, :], wq_sb[:, dc, :], gn_sb[:, dc : dc + 1]
                )
                nc.vector.tensor_scalar_mul(
                    wkv_sb[:, dc, :], wkv_sb[:, dc, :], gn_sb[:, dc : dc + 1]
                )
            # W_k duplicated on both column halves (k^T lands on both
            # partition halves for row-tiled sim matmuls); W_v is a view.
            wkd_sb = consts.tile([128, 4, 128], F32R)
            nc.vector.tensor_copy(wkd_sb[:, :, 0:64], wkv_sb[:, :, 0:64])
            nc.vector.tensor_copy(wkd_sb[:, :, 64:128], wkv_sb[:, :, 0:64])

            ones_f = consts.tile([128, 64], F32)
            nc.vector.memset(ones_f[:], 1.0)
            ones_b = consts.tile([128, 64], BF16)
            nc.vector.tensor_copy(ones_b[:], ones_f[:])

            if apply_gout:
                go_sb = consts.tile([128, DIM], F32)
                go_ap = go_d.ap()
                nc.sync.dma_start(
                    go_sb[:],
                    bass.AP(tensor=go_ap.tensor, offset=go_ap.offset,
                            ap=[[0, 128]] + list(go_ap.ap)),
                )

            x_view = x_d.ap().rearrange("(p2 t p) d -> p2 p t d", p=128, t=4)
            y_view = y_d.ap().rearrange("(p2 t p) d -> p2 p t d", p=128, t=4)

            for pr in range(PAIRS):
                # ---- load + LayerNorm 1 ----
                xin = work.tile([128, 4, DIM], F32)
                nc.sync.dma_start(xin[:], x_view[pr])
                xn = work.tile([128, 4, DIM], F32R)
                for t in range(4):
                    st = statsp.tile([128, 6], F32)
                    nc.vector.bn_stats(st[:], xin[:, t, :])
                    mv = statsp.tile([128, 2], F32)
                    nc.vector.bn_aggr(mv[:], st[:])
                    rstd = statsp.tile([128, 1], F32)
                    nc.scalar.activation(
                        out=rstd[:], in_=mv[:, 1:2], func=AF.Sqrt,
                        bias=eps_t[:], scale=1.0,
                    )
                    nc.vector.reciprocal(out=rstd[:], in_=rstd[:])
                    nc.vector.tensor_scalar(
                        out=xn[:, t, :], in0=xin[:, t, :],
                        scalar1=mv[:, 0:1], scalar2=rstd[:],
                        op0=OP.subtract, op1=OP.mult,
                    )

                # ---- transpose xn -> xnT [dim, tok] ----
                stage = ps.tile([128, 4, 4, 128], F32R, tag="ps")
                for dc in range(4):
                    for t in range(4):
                        nc.tensor.transpose(
                            stage[:, dc, t, :],
                            xn[:, t, dc * 128 : (dc + 1) * 128],
                            ident[:],
                        )
                xnT = work.tile([128, 4, DIM], F32R)
                nc.vector.tensor_copy(xnT[:], stage[:])

                # ---- q^T ----
                qTp = ps.tile([128, 4, DIM], F32, tag="ps")
                for c in range(4):
                    for dc in range(4):
                        nc.tensor.matmul(
                            qTp[:, c, :],
                            wq_sb[:, dc, c * 128 : (c + 1) * 128],
                            xnT[:, dc, :],
                            start=(dc == 0), stop=(dc == 3),
                        )
                qT = work.tile([128, 4, DIM], F32R)
                nc.vector.tensor_copy(qT[:], qTp[:])

                # ---- k^T (duplicated) and v^T, then v ----
                kvp = ps.tile([128, 4, DIM], F32, tag="ps")
                for dc in range(4):
                    nc.tensor.matmul(
                        kvp[:, 0, :], wkd_sb[:, dc, :], xnT[:, dc, :],
                        start=(dc == 0), stop=(dc == 3),
                    )
                for dc in range(4):
                    nc.tensor.matmul(
                        kvp[0:64, 1, :],
                        wkv_sb[:, dc, 64:128],
                        xnT[:, dc, :],
                        start=(dc == 0), stop=(dc == 3),
                    )
                kk = work.tile([128, DIM], F32R)
                nc.vector.tensor_copy(kk[:], kvp[:, 0, :])
                vT = work.tile([128, DIM], F32R)
                nc.vector.tensor_copy(vT[0:64, :], kvp[0:64, 1, :])

                vtp = ps.tile([128, 4, 64], F32R, tag="ps")
                for s in range(4):  # s = g*2 + jc
                    nc.tensor.transpose(
                        vtp[:, s, :],
                        vT[0:64, s * 128 : (s + 1) * 128],
                        ident[0:64, 0:64],
                    )
                v_sb = work.tile([128, 4, 64], BF16)
                nc.vector.tensor_copy(v_sb[:], vtp[:])

                # ---- sim^T + exp per (group, j-chunk) ----
                attnT = [[None, None], [None, None]]
                for g in range(2):
                    for jc in range(2):
                        simp = ps.tile([128, 8, 256], F32, tag="ps")
                        for c in range(4):
                            for hp in range(2):
                                sl = hp * 4 + c
                                nc.tensor.matmul(
                                    simp[:, sl, :],
                                    kk[hp * 64 : hp * 64 + 64,
                                       g * 256 + jc * 128 : g * 256 + jc * 128 + 128],
                                    qT[hp * 64 : hp * 64 + 64, c,
                                       g * 256 : (g + 1) * 256],
                                    start=True, stop=True,
                                    tile_position=(hp * 64, 0),
                                )
                        at = attnp.tile([128, 8, 256], BF16, tag="attnT")
                        nc.scalar.activation(
                            out=at[:], in_=simp[:], func=AF.Exp, scale=EXP_SCALE,
                        )
                        attnT[g][jc] = at

                # ---- attn @ v (transposed out), bf16 col-tiled ----
                avp = ps.tile([128, 2, 4, 256], F32, tag="ps")
                for g in range(2):
                    for ic in range(4):
                        for jc in range(2):
                            for hp in range(2):
                                sl = hp * 4 + ic
                                nc.tensor.matmul(
                                    avp[hp * 64 : hp * 64 + 64, g, ic, :],
                                    v_sb[:, g * 2 + jc, :],
                                    attnT[g][jc][:, sl, :],
                                    start=(jc == 0), stop=(jc == 1),
                                    tile_position=(0, hp * 64),
                                )
                dnp = ps.tile([128, 2, 4, 256], F32, tag="ps")
                for g in range(2):
                    for ic in range(4):
                        for jc in range(2):
                            for hp in range(2):
                                sl = hp * 4 + ic
                                nc.tensor.matmul(
                                    dnp[hp * 64 : hp * 64 + 64, g, ic, :],
                                    ones_b[:, :],
                                    attnT[g][jc][:, sl, :],
                                    start=(jc == 0), stop=(jc == 1),
                                    tile_position=(0, hp * 64),
                                )
                dn_sb = work.tile([128, 2, 4, 256], F32)
                nc.vector.reciprocal(out=dn_sb[:], in_=dnp[:])
                outT = work.tile([128, 4, DIM], F32R)
                for g in range(2):
                    nc.vector.tensor_tensor(
                        outT[:, :, g * 256 : (g + 1) * 256],
                        avp[:, g, :, :], dn_sb[:, g, :, :], OP.mult,
                    )

                # ---- out projection + LayerNorm 2 ----
                finp = ps.tile([128, 4, DIM], F32, tag="ps")
                for t in range(4):
                    for ic in range(4):
                        nc.tensor.matmul(
                            finp[:, t, :],
                            outT[:, ic, t * 128 : (t + 1) * 128],
                            wo_sb[:, ic, :],
                            start=(ic == 0), stop=(ic == 3),
                        )
                y_sb = work.tile([128, 4, DIM], F32)
                for t in range(4):
                    st2 = statsp.tile([128, 6], F32)
                    nc.vector.bn_stats(st2[:], finp[:, t, :])
                    mv2 = statsp.tile([128, 2], F32)
                    nc.vector.bn_aggr(mv2[:], st2[:])
                    rstd2 = statsp.tile([128, 1], F32)
                    nc.scalar.activation(
                        out=rstd2[:], in_=mv2[:, 1:2], func=AF.Sqrt,
                        bias=eps_t[:], scale=1.0,
                    )
                    nc.vector.reciprocal(out=rstd2[:], in_=rstd2[:])
                    nc.vector.tensor_scalar(
                        out=y_sb[:, t, :], in0=finp[:, t, :],
                        scalar1=mv2[:, 0:1], scalar2=rstd2[:],
                        op0=OP.subtract, op1=OP.mult,
                    )
                    if apply_gout:
                        nc.vector.tensor_tensor(
                            y_sb[:, t, :], y_sb[:, t, :], go_sb[:], OP.mult
                        )
                nc.sync.dma_start(y_view[pr], y_sb[:])
                if debug and pr == 0:
                    nc.sync.dma_start(dbg["xn"].ap(), xn[:])
                    nc.sync.dma_start(dbg["xnT"].ap(), xnT[:])
                    nc.sync.dma_start(dbg["qT"].ap(), qT[:])
                    nc.sync.dma_start(dbg["kk"].ap(), kk[:])
                    nc.sync.dma_start(dbg["vT"].ap(), vT[:])
                    nc.sync.dma_start(dbg["v"].ap(), v_sb[:])
                    for g in range(2):
                        for jc in range(2):
                            nc.sync.dma_start(dbg["at"].ap()[g * 2 + jc], attnT[g][jc][:])
                    nc.sync.dma_start(dbg["outT"].ap(), outT[:])

    nc.finalize()
    return nc


def kernel(x, g_norm, W_q, W_kv, W_out, g_out):
    from concourse.bass_utils import run_bass_kernel_spmd

    x = np.ascontiguousarray(np.asarray(x, dtype=np.float32))
    g_norm = np.asarray(g_norm, dtype=np.float32)
    W_q = np.ascontiguousarray(np.asarray(W_q, dtype=np.float32))
    W_kv = np.ascontiguousarray(np.asarray(W_kv, dtype=np.float32))
    W_out = np.ascontiguousarray(np.asarray(W_out, dtype=np.float32))
    g_out = np.asarray(g_out, dtype=np.float32)

    apply_gout = not np.allclose(g_out, 1.0)
    key = apply_gout
    if key not in _BUILD_CACHE:
        _BUILD_CACHE[key] = _build_nc(apply_gout)
    nc = _BUILD_CACHE[key]

    xs = x.reshape(GROUPS, R, DIM)
    in_maps = []
    for c in range(NCORES):
        in_maps.append({
            "x": np.ascontiguousarray(
                xs[c * GPC : (c + 1) * GPC].reshape(GPC * R, DIM)
            ),
            "g_norm": g_norm,
            "W_q": W_q,
            "W_kv": W_kv,
            "W_out": W_out,
            "g_out": g_out,
        })

    res = run_bass_kernel_spmd(nc, in_maps, core_ids=list(range(NCORES)))
    y = np.empty((GROUPS, R, DIM), dtype=np.float32)
    for c in range(NCORES):
        y[c * GPC : (c + 1) * GPC] = res.results[c]["y"].reshape(GPC, R, DIM)
    return y.reshape(B, N, R, DIM)
